# revision 5
# baseline (speedup 1.0000x reference)
"""ContinuousMask kernel for Trainium2 (8 NeuronCores, SPMD row-sharded).

Problem: starts[B=2048, N=8192] int32, T=16384, l=1638. Output bool [B, T]:
True everywhere except the union of windows [s, s+l) over each row's starts.

Algorithm (per row):
  A position t is covered iff some start lies in (t-l, t]. With value-chunks
  of width W=512 (2W <= l), if every chunk 0..(smax>>9)-1 contains at least
  one start, then the covered region is EXACTLY [smin, smax+l):
    - t in [smin, smin+l): covered by the smin window.
    - t in [smin+l, smax): the previous chunk of t is nonempty; any start s'
      there satisfies t-l < s' <= t (since 2W <= l).
    - t in [smax, smax+l): covered by the smax window.
    - t < smin or t >= smax+l: no start in (t-l, t].
  The device computes smin, smax (reduces) and an exact 29-bit chunk
  occupancy bitmask (shift, 1<<hi, tree bitwise-or), flags rows where the
  occupancy condition fails (or where smin/smax fall outside the painted
  strips), and paints the mask from smin/smax. Flagged rows (probability
  ~exp(-284) per chunk under the problem's distribution, i.e. never) are
  recomputed exactly on host.
"""

import numpy as np

B = 2048
T = 16384
NSEG = 8192
L = 1638
NCORES = 8
RPC = B // NCORES  # 256 rows per core
PT = 128  # rows per partition tile
NRT = RPC // PT  # 2 row tiles per core
CHUNK = 2048  # starts columns per DMA chunk
NCK = NSEG // CHUNK  # 4
SHIFT = 9  # occupancy chunk width 512 (2*512 <= L)
HSTRIP = 2048  # head strip [0, HSTRIP)
TSTART = T - 2048  # tail strip [TSTART, T)
SMAX_MIN = TSTART - L  # flag row if smax < this (tail True-run would start left of strip)

_prog_cache: dict = {}


def _build_program():
    import concourse.bacc as bacc
    import concourse.mybir as mybir
    from concourse.tile import TileContext

    dt = mybir.dt
    Alu = mybir.AluOpType
    X = mybir.AxisListType.X

    nc = bacc.Bacc("TRN2", debug=False)
    starts_d = nc.declare_dram_parameter("starts", [RPC, NSEG], dt.int32, isOutput=False)
    mask_d = nc.declare_dram_parameter("mask", [RPC, T], dt.uint8, isOutput=True)
    flags_d = nc.declare_dram_parameter("flags", [RPC, 1], dt.int32, isOutput=True)

    with TileContext(nc) as tc:
        with (
            tc.tile_pool(name="persist", bufs=1) as pp,
            tc.tile_pool(name="work", bufs=3) as wp,
            tc.tile_pool(name="small", bufs=4) as sp,
        ):
            iota_t = pp.tile([PT, HSTRIP], dt.int16, tag="iota")
            nc.gpsimd.iota(iota_t[:], [[1, HSTRIP]], base=0, channel_multiplier=0)
            ones_t = pp.tile([PT, CHUNK], dt.int32, tag="ones")
            nc.vector.memset(ones_t[:], 1)
            neg1_t = pp.tile([PT, 1], dt.int32, tag="neg1")
            nc.vector.memset(neg1_t[:], -1)

            out_tiles = []
            for rt in range(NRT):
                ot = pp.tile([PT, T], dt.uint8, tag=f"out{rt}")
                nc.gpsimd.memset(ot[:], 0)
                out_tiles.append(ot)

            for rt in range(NRT):
                r0 = rt * PT
                mins, maxs, occs = [], [], []
                for ck in range(NCK):
                    st = wp.tile([PT, CHUNK], dt.int32, tag="st")
                    nc.sync.dma_start(
                        out=st[:],
                        in_=starts_d[r0 : r0 + PT, ck * CHUNK : (ck + 1) * CHUNK],
                    )
                    mn = sp.tile([PT, 1], dt.int32, tag="mn")
                    mx = sp.tile([PT, 1], dt.int32, tag="mx")
                    nc.vector.tensor_reduce(mn[:], st[:], X, Alu.min)
                    nc.vector.tensor_reduce(mx[:], st[:], X, Alu.max)
                    hi = wp.tile([PT, CHUNK], dt.int32, tag="hi")
                    nc.vector.tensor_scalar(hi[:], st[:], SHIFT, None, Alu.arith_shift_right)
                    bits = wp.tile([PT, CHUNK], dt.int32, tag="bits")
                    nc.vector.tensor_tensor(bits[:], ones_t[:], hi[:], Alu.logical_shift_left)
                    w = CHUNK
                    while w > 1:
                        h = w // 2
                        nc.vector.tensor_tensor(
                            bits[:, 0:h], bits[:, 0:h], bits[:, h:w], Alu.bitwise_or
                        )
                        w = h
                    occ1 = sp.tile([PT, 1], dt.int32, tag="occ1")
                    nc.vector.tensor_copy(occ1[:], bits[:, 0:1])
                    mins.append(mn)
                    maxs.append(mx)
                    occs.append(occ1)

                # combine partials -> smin, smax, occ  [PT, 1] each
                while len(mins) > 1:
                    nc.vector.tensor_tensor(mins[0][:], mins[0][:], mins.pop()[:], Alu.min)
                    nc.vector.tensor_tensor(maxs[0][:], maxs[0][:], maxs.pop()[:], Alu.max)
                    nc.vector.tensor_tensor(occs[0][:], occs[0][:], occs.pop()[:], Alu.bitwise_or)
                smin = mins[0]
                smax = maxs[0]
                occ = occs[0]

                # qfail iff occ is missing a bit below (smax >> SHIFT):
                #   (occ | (-1 << clast)) != -1   (pure bitwise; fp32-safe compare)
                clast = sp.tile([PT, 1], dt.int32, tag="clast")
                nc.vector.tensor_scalar(clast[:], smax[:], SHIFT, None, Alu.arith_shift_right)
                negm = sp.tile([PT, 1], dt.int32, tag="negm")
                nc.vector.tensor_tensor(negm[:], neg1_t[:], clast[:], Alu.logical_shift_left)
                qa = sp.tile([PT, 1], dt.int32, tag="qa")
                nc.vector.tensor_tensor(qa[:], occ[:], negm[:], Alu.bitwise_or)
                bad = sp.tile([PT, 1], dt.int32, tag="bad")
                nc.vector.tensor_scalar(bad[:], qa[:], -1.0, None, Alu.not_equal)
                # strip-reach guards
                b2 = sp.tile([PT, 1], dt.int32, tag="b2")
                nc.vector.tensor_scalar(b2[:], smax[:], float(SMAX_MIN), None, Alu.is_lt)
                nc.vector.tensor_tensor(bad[:], bad[:], b2[:], Alu.logical_or)
                b3 = sp.tile([PT, 1], dt.int32, tag="b3")
                nc.vector.tensor_scalar(b3[:], smin[:], float(HSTRIP), None, Alu.is_ge)
                nc.vector.tensor_tensor(bad[:], bad[:], b3[:], Alu.logical_or)
                nc.sync.dma_start(out=flags_d[r0 : r0 + PT, :], in_=bad[:])

                # paint strips
                smin_f = sp.tile([PT, 1], dt.float32, tag="sminf")
                nc.vector.tensor_copy(smin_f[:], smin[:])
                smaxl_f = sp.tile([PT, 1], dt.float32, tag="smaxlf")
                nc.vector.tensor_scalar(smaxl_f[:], smax[:], float(L - TSTART), None, Alu.add)
                ot = out_tiles[rt]
                nc.vector.tensor_scalar(ot[:, 0:HSTRIP], iota_t[:], smin_f[:], None, Alu.is_lt)
                nc.vector.tensor_scalar(ot[:, TSTART:T], iota_t[:], smaxl_f[:], None, Alu.is_ge)
                nc.sync.dma_start(out=mask_d[r0 : r0 + PT, :], in_=ot[:])

    nc.finalize()
    return nc


def _get_program():
    if "nc" not in _prog_cache:
        _prog_cache["nc"] = _build_program()
    return _prog_cache["nc"]


def _host_exact_row(row_starts: np.ndarray) -> np.ndarray:
    delta = np.zeros(T + 1, np.int64)
    np.add.at(delta, row_starts, 1)
    np.add.at(delta, row_starts + L, -1)
    return ~(np.cumsum(delta)[:T] > 0)


def run_device(starts: np.ndarray, trace: bool = False):
    """Run the SPMD bass kernel. Returns (mask_u8 [B,T], flags [B], results)."""
    from concourse.bass_utils import run_bass_kernel_spmd

    nc = _get_program()
    shards = starts.reshape(NCORES, RPC, NSEG)
    in_maps = [{"starts": np.ascontiguousarray(shards[c])} for c in range(NCORES)]
    res = run_bass_kernel_spmd(nc, in_maps, list(range(NCORES)), trace=trace)
    mask = np.concatenate([r["mask"] for r in res.results], axis=0)
    flags = np.concatenate([r["flags"] for r in res.results], axis=0).reshape(-1)
    return mask, flags, res


def kernel(**inputs) -> np.ndarray:
    starts = np.ascontiguousarray(np.asarray(inputs["starts"]), dtype=np.int32)
    t_in = int(np.asarray(inputs["T"]))
    l_in = int(np.asarray(inputs["l"]))
    assert starts.shape == (B, NSEG), starts.shape
    assert t_in == T and l_in == L, (t_in, l_in)

    mask_u8, flags, _ = run_device(starts)
    mask = mask_u8.astype(bool)

    bad_rows = np.nonzero(flags != 0)[0]
    for r in bad_rows:  # pathological rows: exact host recompute (never on real data)
        mask[r] = _host_exact_row(starts[r])
    return mask
